# revision 8
# baseline (speedup 1.0000x reference)
"""ChebConv (complex, K+1=3 hops) Trainium2 kernel over 8 NeuronCores.

Sharding: 1D node partition on destination rows (6250 rows/core), full X
replicated in each core's HBM; each core processes exactly the edges
targeting its rows, so no collectives are needed.

Per core, edges are bucketed by (batch of 6 row-groups, col-half, group)
and padded to 128-edge blocks with a layout shared across cores (max block
count per bucket). Per batch bt:

  pk   = one packed [128, nbt*15] int16 stream: gather indices (wrapped
         16-part layout, 8 cols/block), c6 vals (6 bf16/block) and local
         row jl (1 bf16/block)
  gt   = TWO dma_gather calls (one per col half) fetch all nbt blocks'
         neighbor rows ([128, 512] bf16 each) in one go — the ~1us fixed
         SWDGE cost is paid 2x per batch instead of 2x per group
  V    = built for the whole batch in 2 DVE ops (broadcast is_equal
         against the j-iota, broadcast multiply by c6)
  P_g  = per group, 4 PE matmul accumulation chains (feature chunks q)
         over the group's blocks: P[q] += G_chunk.T @ V_block
  out  = per batch, 24 PE matmuls contract P with signed weight tiles;
         bias added during PSUM->SBUF copy on DVE
"""
import sys
sys.path.insert(0, '/opt/trn_rl_repo')

import numpy as np
import ml_dtypes

N = 50000
E = 1_600_000
K1 = 3
C = 256
CORES = 8
RPC = N // CORES            # 6250 rows per core
GR = 21                     # rows per group
MCOLS = 6 * GR              # 126 one-hot columns
GPB = 6                     # groups per batch
ROWS_PB = GR * GPB          # 126
NB = -(-RPC // ROWS_PB)     # 50 batches
REAL_GRP = -(-RPC // GR)    # 298
HALF = 32768
NQ = 4                      # SWDGE queues


def _bf16(x):
    return x.astype(ml_dtypes.bfloat16)


def _preprocess(rows, cols, Lr, Li, weight, bias):
    rows = np.asarray(rows).astype(np.int64)
    cols = np.asarray(cols).astype(np.int64)
    core = rows // RPC
    rloc = rows - core * RPC
    g = rloc // GR
    bt = g // GPB
    gl = g - bt * GPB
    jl = (rloc - g * GR).astype(np.float32)

    C6 = np.empty((E, 6), np.float32)
    C6[:, 0:3] = np.asarray(Lr).T
    C6[:, 3:6] = np.asarray(Li).T

    colh = (cols >= HALF).astype(np.int64)
    key = ((core * NB + bt) * 2 + colh) * GPB + gl
    order = np.argsort(key, kind="stable")
    key_s = key[order]
    nbuck = CORES * NB * 2 * GPB
    bounds = np.searchsorted(key_s, np.arange(nbuck + 1))
    cnt = (bounds[1:] - bounds[:-1]).reshape(CORES, NB, 2, GPB)

    # shared block counts per (batch, half, group) = max over cores
    nblk = -(-cnt.max(axis=0) // 128)             # [NB, 2, GPB]
    for b in range(NB):
        for l in range(GPB):
            if b * GPB + l < REAL_GRP and nblk[b, :, l].sum() == 0:
                nblk[b, 0, l] = 1

    nbt = nblk.sum(axis=(1, 2))                   # blocks per batch
    tot_blk = int(nbt.sum())
    # per-batch metadata
    batches = []                                  # per bt: dict
    pk_off = np.zeros(NB + 1, np.int64)
    blk_abs = 0
    for b in range(NB):
        pk_off[b + 1] = pk_off[b] + nbt[b] * 16
        halves = []                               # (h, rel_off, nbh)
        gblocks = [[] for _ in range(GPB)]        # rel block ids per group
        fills = []                                # (h, gl, rel_b, nb)
        rel = 0
        for h in range(2):
            nbh = int(nblk[b, h].sum())
            if nbh:
                halves.append((h, rel, nbh))
            for l in range(GPB):
                nbl = int(nblk[b, h, l])
                if nbl:
                    gblocks[l].extend(range(rel, rel + nbl))
                    fills.append((h, l, rel, nbl))
                    rel += nbl
        assert rel == nbt[b]
        batches.append(dict(nbt=int(nbt[b]), halves=halves,
                            gblocks=gblocks, fills=fills, abs0=blk_abs))
        blk_abs += int(nbt[b])
    tot_pack = int(pk_off[NB])

    cols_s = cols[order]
    C6_s = _bf16(C6[order]).view(np.uint16)
    jl_s = jl[order].astype(np.float32).reshape(-1, 1).view(np.uint16)
    per_core = []
    for c in range(CORES):
        pk = np.zeros((128, tot_pack), np.uint16)
        for b in range(NB):
            nb_b = batches[b]["nbt"]
            base = int(pk_off[b])
            idx16 = np.zeros(nb_b * 128, np.int16)
            c6t = np.zeros((128, nb_b * 6), np.uint16)
            jlf = np.zeros((128, nb_b * 2), np.uint16)
            for h, l, rel_b, nbl in batches[b]["fills"]:
                buck = ((c * NB + b) * 2 + h) * GPB + l
                lo, hi = int(bounds[buck]), int(bounds[buck + 1])
                ne = hi - lo
                if ne == 0:
                    continue
                idx16[rel_b * 128:rel_b * 128 + ne] = \
                    (cols_s[lo:hi] - h * HALF).astype(np.int16)
                cc = C6_s[lo:hi]
                jj = jl_s[lo:hi]
                for k in range(nbl):
                    a, e = k * 128, min((k + 1) * 128, ne)
                    if a >= e:
                        break
                    c6t[0:e - a, (rel_b + k) * 6:(rel_b + k) * 6 + 6] = cc[a:e]
                    jlf[0:e - a, (rel_b + k) * 2:(rel_b + k) * 2 + 2] = jj[a:e]
            idxw = np.tile(idx16.reshape(-1, 16).T, (8, 1))  # [128, nb_b*8]
            pk[:, base:base + nb_b * 8] = idxw.view(np.uint16)
            pk[:, base + nb_b * 8:base + nb_b * 14] = c6t
            pk[:, base + nb_b * 14:base + nb_b * 16] = jlf
        per_core.append(dict(pk=np.ascontiguousarray(pk.view(np.int16))))

    # weight tiles [12][128, 256] f32: 0..5 = +W[k][fh], 6..11 = -W[k][fh]
    weight = np.asarray(weight, np.float32)
    wt = np.empty((12, 128, C), np.float32)
    for fh in range(2):
        for k in range(K1):
            wt[fh * 3 + k] = weight[k][fh * 128:(fh + 1) * 128]
            wt[6 + fh * 3 + k] = -weight[k][fh * 128:(fh + 1) * 128]
    wsb = np.ascontiguousarray(wt.transpose(1, 0, 2).reshape(128, 12 * C))

    biasr = np.ascontiguousarray(np.tile(np.asarray(bias, np.float32), (128, 1)))
    # V column layout per block: m = s*21 + j  ->  j = m % 21
    mdiv6 = np.ascontiguousarray(
        _bf16(np.tile((np.arange(MCOLS) % GR).astype(np.float32), (128, 1))))

    return dict(tot_pack=tot_pack, pk_off=pk_off, batches=batches,
                per_core=per_core, wsb=wsb, biasr=biasr, mdiv6=mdiv6)


def _final_mm_list():
    """(target, q, s, wtile): target 0=real 1=imag; q = P region; s = slot."""
    mms = []
    for tgt in range(2):
        for fh in range(2):
            for k in range(K1):
                if tgt == 0:
                    mms.append((0, fh, k, fh * 3 + k))               # +W  P_r
                    mms.append((0, 2 + fh, 3 + k, 6 + fh * 3 + k))   # -W  P_i
                else:
                    mms.append((1, fh, 3 + k, fh * 3 + k))           # +W  P_r
                    mms.append((1, 2 + fh, k, fh * 3 + k))           # +W  P_i
    return mms


def _build(nc, prep, repeat=1):
    import concourse.mybir as mybir
    from concourse.tile import TileContext
    import contextlib

    f32 = mybir.dt.float32
    bf16 = mybir.dt.bfloat16
    i16 = mybir.dt.int16
    tot_pack = prep["tot_pack"]
    pk_off = prep["pk_off"]
    batches = prep["batches"]

    xcat = nc.dram_tensor("xcat", [N, 512], bf16, kind="ExternalInput")
    pk_d = nc.dram_tensor("pk", [128, tot_pack], i16, kind="ExternalInput")
    w_d = nc.dram_tensor("wt", [128, 12 * C], f32, kind="ExternalInput")
    bias_d = nc.dram_tensor("biasr", [128, C], f32, kind="ExternalInput")
    md_d = nc.dram_tensor("mdiv6", [128, MCOLS], bf16, kind="ExternalInput")
    or_d = nc.dram_tensor("out_r", [NB * ROWS_PB, C], f32, kind="ExternalOutput")
    oi_d = nc.dram_tensor("out_i", [NB * ROWS_PB, C], f32, kind="ExternalOutput")

    mms = _final_mm_list()

    with TileContext(nc) as tc:
        with tc.tile_pool(name="const", bufs=1) as cpool, \
             tc.tile_pool(name="pk", bufs=3) as pkpool, \
             tc.tile_pool(name="g", bufs=2) as gpool, \
             tc.tile_pool(name="v", bufs=2) as vpool, \
             tc.tile_pool(name="pb", bufs=2) as pbpool, \
             tc.tile_pool(name="os", bufs=4) as ospool, \
             tc.tile_pool(name="ps", bufs=4, space="PSUM") as pspool, \
             tc.tile_pool(name="po", bufs=2, space="PSUM") as popool:

            w_t = cpool.tile([128, 12 * C], f32)
            bias_t = cpool.tile([128, C], f32)
            md_t = cpool.tile([128, MCOLS], bf16)
            for dst, src in [(w_t, w_d), (bias_t, bias_d), (md_t, md_d)]:
                nc.sync.dma_start(dst[:], src[:])

            rep_cm = tc.For_i(0, repeat, 1) if repeat > 1 else contextlib.nullcontext()
            with rep_cm:
              qn = 0
              for b in range(NB):
                bm = batches[b]
                nbt = bm["nbt"]
                pk_t = pkpool.tile([128, nbt * 16], i16, tag="pk")
                nc.sync.dma_start(
                    pk_t[:], pk_d[:, int(pk_off[b]):int(pk_off[b]) + nbt * 16])

                gt = gpool.tile([128, nbt * 512], bf16, tag="g")
                for h, rel, nbh in bm["halves"]:
                    src = xcat[:] if h == 0 else xcat[HALF:, :]
                    nc.gpsimd.dma_gather(
                        gt[:, rel * 512:(rel + nbh) * 512]
                          .rearrange("p (b e) -> p b e", e=512),
                        src,
                        pk_t[:, rel * 8:(rel + nbh) * 8],
                        nbh * 128, nbh * 128, 512,
                        queue_num=qn, single_packet=False,
                    )
                    qn = (qn + 1) % NQ

                v_t = vpool.tile([128, nbt * MCOLS], bf16, tag="v")
                jl_ap = pk_t[:, nbt * 14:nbt * 16].bitcast(f32)
                c6_ap = pk_t[:, nbt * 8:nbt * 14].bitcast(bf16)
                for blk in range(nbt):
                    nc.vector.tensor_scalar(
                        v_t[:, blk * MCOLS:(blk + 1) * MCOLS], md_t[:],
                        jl_ap[:, blk:blk + 1], None,
                        mybir.AluOpType.is_equal)
                    c6rep = c6_ap[:, blk * 6:blk * 6 + 6] \
                        .unsqueeze(2).broadcast_to((128, 6, GR))
                    nc.vector.tensor_tensor(
                        v_t[:, blk * MCOLS:(blk + 1) * MCOLS]
                            .rearrange("p (s x) -> p s x", x=GR),
                        v_t[:, blk * MCOLS:(blk + 1) * MCOLS]
                            .rearrange("p (s x) -> p s x", x=GR),
                        c6rep, mybir.AluOpType.mult)

                pbuf = pbpool.tile([128, GPB * 504], f32, tag="pbuf")
                for gl in range(GPB):
                    blocks = bm["gblocks"][gl]
                    if not blocks:
                        nc.vector.memset(
                            pbuf[:].rearrange(
                                "p (q s g j) -> p q s g j", q=4, s=6, g=GPB)[
                                :, :, :, gl, :], 0.0)
                        continue
                    p_t = pspool.tile([128, 504], f32, tag="p")
                    nlast = len(blocks) - 1
                    for q in range(4):
                        for i, blk in enumerate(blocks):
                            nc.tensor.matmul(
                                p_t[:, q * 126:(q + 1) * 126],
                                gt[:, blk * 512 + q * 128:blk * 512 + (q + 1) * 128],
                                v_t[:, blk * 126:(blk + 1) * 126],
                                start=(i == 0), stop=(i == nlast))
                    pb_dst = pbuf[:].rearrange(
                        "p (q s g j) -> p q s g j", q=4, s=6, g=GPB)[
                        :, :, :, gl, :]
                    nc.scalar.copy(pb_dst, p_t[:])

                po_r = popool.tile([128, C], f32, tag="por")
                po_i = popool.tile([128, C], f32, tag="poi")
                nmm = {0: 0, 1: 0}
                for tgt, q, s, wi in mms:
                    po = po_r if tgt == 0 else po_i
                    plane = q * 6 + s
                    lhsT = pbuf[:, plane * MCOLS:(plane + 1) * MCOLS]
                    nc.tensor.matmul(
                        po[:MCOLS, :], lhsT, w_t[:, wi * C:(wi + 1) * C],
                        start=(nmm[tgt] == 0), stop=(nmm[tgt] == 11))
                    nmm[tgt] += 1
                o_r = ospool.tile([128, C], f32, tag="or")
                o_i = ospool.tile([128, C], f32, tag="oi")
                nc.vector.tensor_tensor(o_r[:MCOLS, :], po_r[:MCOLS, :],
                                        bias_t[:MCOLS, :], mybir.AluOpType.add)
                nc.vector.tensor_tensor(o_i[:MCOLS, :], po_i[:MCOLS, :],
                                        bias_t[:MCOLS, :], mybir.AluOpType.add)
                nc.sync.dma_start(or_d[b * ROWS_PB:(b + 1) * ROWS_PB, :],
                                   o_r[:MCOLS, :])
                nc.sync.dma_start(oi_d[b * ROWS_PB:(b + 1) * ROWS_PB, :],
                                  o_i[:MCOLS, :])


def _make_nc(prep, repeat=1):
    import concourse.bacc as bacc
    nc = bacc.Bacc("TRN2", target_bir_lowering=False, debug=False,
                   num_swdge_queues=NQ)
    _build(nc, prep, repeat=repeat)
    nc.compile()
    return nc


def _in_maps(prep, X_real, X_imag):
    xcat = _bf16(np.concatenate(
        [np.asarray(X_real, np.float32), np.asarray(X_imag, np.float32)],
        axis=1))
    maps = []
    for c in range(CORES):
        maps.append({
            "xcat": xcat, "pk": prep["per_core"][c]["pk"],
            "wt": prep["wsb"], "biasr": prep["biasr"], "mdiv6": prep["mdiv6"],
        })
    return maps


def kernel(X_real, X_imag, L_real_vals, L_imag_vals, weight, bias, rows, cols):
    from concourse.bass_utils import run_bass_kernel_spmd

    prep = _preprocess(rows, cols, L_real_vals, L_imag_vals, weight, bias)
    nc = _make_nc(prep)
    res = run_bass_kernel_spmd(nc, _in_maps(prep, X_real, X_imag),
                               core_ids=list(range(CORES)))
    out_r = np.concatenate([res.results[c]["out_r"][:RPC] for c in range(CORES)], 0)
    out_i = np.concatenate([res.results[c]["out_i"][:RPC] for c in range(CORES)], 0)
    return out_r, out_i


# revision 9
# speedup vs baseline: 1.0457x; 1.0457x over previous
"""ChebConv (complex, K+1=3 hops) Trainium2 kernel over 8 NeuronCores.

Sharding: 1D node partition on destination rows (6250 rows/core), full X
replicated in each core's HBM; each core processes exactly the edges
targeting its rows, so no collectives are needed.

Per core, edges are bucketed by (batch of 6 row-groups, col-half, group)
and padded to 128-edge blocks with a layout shared across cores (max block
count per bucket). Per batch bt:

  pk   = one packed [128, nbt*15] int16 stream: gather indices (wrapped
         16-part layout, 8 cols/block), c6 vals (6 bf16/block) and local
         row jl (1 bf16/block)
  gt   = TWO dma_gather calls (one per col half) fetch all nbt blocks'
         neighbor rows ([128, 512] bf16 each) in one go — the ~1us fixed
         SWDGE cost is paid 2x per batch instead of 2x per group
  V    = built for the whole batch in 2 DVE ops (broadcast is_equal
         against the j-iota, broadcast multiply by c6)
  P_g  = per group, 4 PE matmul accumulation chains (feature chunks q)
         over the group's blocks: P[q] += G_chunk.T @ V_block
  out  = per batch, 24 PE matmuls contract P with signed weight tiles;
         bias added during PSUM->SBUF copy on DVE
"""
import sys
sys.path.insert(0, '/opt/trn_rl_repo')

import numpy as np
import ml_dtypes

N = 50000
E = 1_600_000
K1 = 3
C = 256
CORES = 8
RPC = N // CORES            # 6250 rows per core
GR = 21                     # rows per group
MCOLS = 6 * GR              # 126 one-hot columns
GPB = 6                     # groups per batch
ROWS_PB = GR * GPB          # 126
NB = -(-RPC // ROWS_PB)     # 50 batches
REAL_GRP = -(-RPC // GR)    # 298
HALF = 32768
NQ = 4                      # SWDGE queues


def _bf16(x):
    return x.astype(ml_dtypes.bfloat16)


def _preprocess(rows, cols, Lr, Li, weight, bias):
    rows = np.asarray(rows).astype(np.int64)
    cols = np.asarray(cols).astype(np.int64)
    core = rows // RPC
    rloc = rows - core * RPC
    g = rloc // GR
    bt = g // GPB
    gl = g - bt * GPB
    jl = (rloc - g * GR).astype(np.float32)

    C6 = np.empty((E, 6), np.float32)
    C6[:, 0:3] = np.asarray(Lr).T
    C6[:, 3:6] = np.asarray(Li).T

    colh = (cols >= HALF).astype(np.int64)
    key = ((core * NB + bt) * 2 + colh) * GPB + gl
    order = np.argsort(key, kind="stable")
    key_s = key[order]
    nbuck = CORES * NB * 2 * GPB
    bounds = np.searchsorted(key_s, np.arange(nbuck + 1))
    cnt = (bounds[1:] - bounds[:-1]).reshape(CORES, NB, 2, GPB)

    # shared block counts per (batch, half, group) = max over cores
    nblk = -(-cnt.max(axis=0) // 128)             # [NB, 2, GPB]
    for b in range(NB):
        for l in range(GPB):
            if b * GPB + l < REAL_GRP and nblk[b, :, l].sum() == 0:
                nblk[b, 0, l] = 1

    nbt = nblk.sum(axis=(1, 2))                   # blocks per batch
    tot_blk = int(nbt.sum())
    # per-batch metadata
    batches = []                                  # per bt: dict
    pk_off = np.zeros(NB + 1, np.int64)
    blk_abs = 0
    for b in range(NB):
        pk_off[b + 1] = pk_off[b] + nbt[b] * 16
        halves = []                               # (h, rel_off, nbh)
        gblocks = [[] for _ in range(GPB)]        # rel block ids per group
        fills = []                                # (h, gl, rel_b, nb)
        rel = 0
        for h in range(2):
            nbh = int(nblk[b, h].sum())
            if nbh:
                halves.append((h, rel, nbh))
            for l in range(GPB):
                nbl = int(nblk[b, h, l])
                if nbl:
                    gblocks[l].extend(range(rel, rel + nbl))
                    fills.append((h, l, rel, nbl))
                    rel += nbl
        assert rel == nbt[b]
        batches.append(dict(nbt=int(nbt[b]), halves=halves,
                            gblocks=gblocks, fills=fills, abs0=blk_abs))
        blk_abs += int(nbt[b])
    tot_pack = int(pk_off[NB])

    cols_s = cols[order]
    C6_s = _bf16(C6[order]).view(np.uint16)
    jl_s = jl[order].astype(np.float32).reshape(-1, 1).view(np.uint16)
    per_core = []
    for c in range(CORES):
        pk = np.zeros((128, tot_pack), np.uint16)
        for b in range(NB):
            nb_b = batches[b]["nbt"]
            base = int(pk_off[b])
            idx16 = np.zeros(nb_b * 128, np.int16)
            c6t = np.zeros((128, nb_b * 6), np.uint16)
            jlf = np.zeros((128, nb_b * 2), np.uint16)
            for h, l, rel_b, nbl in batches[b]["fills"]:
                buck = ((c * NB + b) * 2 + h) * GPB + l
                lo, hi = int(bounds[buck]), int(bounds[buck + 1])
                ne = hi - lo
                if ne == 0:
                    continue
                idx16[rel_b * 128:rel_b * 128 + ne] = \
                    (cols_s[lo:hi] - h * HALF).astype(np.int16)
                cc = C6_s[lo:hi]
                jj = jl_s[lo:hi]
                for k in range(nbl):
                    a, e = k * 128, min((k + 1) * 128, ne)
                    if a >= e:
                        break
                    c6t[0:e - a, (rel_b + k) * 6:(rel_b + k) * 6 + 6] = cc[a:e]
                    jlf[0:e - a, (rel_b + k) * 2:(rel_b + k) * 2 + 2] = jj[a:e]
            idxw = np.tile(idx16.reshape(-1, 16).T, (8, 1))  # [128, nb_b*8]
            pk[:, base:base + nb_b * 8] = idxw.view(np.uint16)
            pk[:, base + nb_b * 8:base + nb_b * 14] = c6t
            pk[:, base + nb_b * 14:base + nb_b * 16] = jlf
        per_core.append(dict(pk=np.ascontiguousarray(pk.view(np.int16))))

    # weight tiles [12][128, 256] f32: 0..5 = +W[k][fh], 6..11 = -W[k][fh]
    weight = np.asarray(weight, np.float32)
    wt = np.empty((12, 128, C), np.float32)
    for fh in range(2):
        for k in range(K1):
            wt[fh * 3 + k] = weight[k][fh * 128:(fh + 1) * 128]
            wt[6 + fh * 3 + k] = -weight[k][fh * 128:(fh + 1) * 128]
    wsb = np.ascontiguousarray(wt.transpose(1, 0, 2).reshape(128, 12 * C))

    biasr = np.ascontiguousarray(np.tile(np.asarray(bias, np.float32), (128, 1)))
    # V column layout per block: m = s*21 + j  ->  j = m % 21
    mdiv6 = np.ascontiguousarray(
        _bf16(np.tile((np.arange(MCOLS) % GR).astype(np.float32), (128, 1))))

    return dict(tot_pack=tot_pack, pk_off=pk_off, batches=batches,
                per_core=per_core, wsb=wsb, biasr=biasr, mdiv6=mdiv6)


def _final_mm_list():
    """(target, q, s, wtile): target 0=real 1=imag; q = P region; s = slot."""
    mms = []
    for tgt in range(2):
        for fh in range(2):
            for k in range(K1):
                if tgt == 0:
                    mms.append((0, fh, k, fh * 3 + k))               # +W  P_r
                    mms.append((0, 2 + fh, 3 + k, 6 + fh * 3 + k))   # -W  P_i
                else:
                    mms.append((1, fh, 3 + k, fh * 3 + k))           # +W  P_r
                    mms.append((1, 2 + fh, k, fh * 3 + k))           # +W  P_i
    return mms


def _build(nc, prep, repeat=1):
    import concourse.mybir as mybir
    from concourse.tile import TileContext
    import contextlib

    f32 = mybir.dt.float32
    bf16 = mybir.dt.bfloat16
    i16 = mybir.dt.int16
    tot_pack = prep["tot_pack"]
    pk_off = prep["pk_off"]
    batches = prep["batches"]

    xcat = nc.dram_tensor("xcat", [N, 512], bf16, kind="ExternalInput")
    pk_d = nc.dram_tensor("pk", [128, tot_pack], i16, kind="ExternalInput")
    w_d = nc.dram_tensor("wt", [128, 12 * C], f32, kind="ExternalInput")
    bias_d = nc.dram_tensor("biasr", [128, C], f32, kind="ExternalInput")
    md_d = nc.dram_tensor("mdiv6", [128, MCOLS], bf16, kind="ExternalInput")
    or_d = nc.dram_tensor("out_r", [NB * ROWS_PB, C], f32, kind="ExternalOutput")
    oi_d = nc.dram_tensor("out_i", [NB * ROWS_PB, C], f32, kind="ExternalOutput")

    mms = _final_mm_list()

    with TileContext(nc) as tc:
        with tc.tile_pool(name="const", bufs=1) as cpool, \
             tc.tile_pool(name="pk", bufs=3) as pkpool, \
             tc.tile_pool(name="g", bufs=2) as gpool, \
             tc.tile_pool(name="v", bufs=2) as vpool, \
             tc.tile_pool(name="pb", bufs=2) as pbpool, \
             tc.tile_pool(name="os", bufs=4) as ospool, \
             tc.tile_pool(name="ps", bufs=4, space="PSUM") as pspool, \
             tc.tile_pool(name="po", bufs=2, space="PSUM") as popool:

            w_t = cpool.tile([128, 12 * C], f32)
            bias_t = cpool.tile([128, C], f32)
            md_t = cpool.tile([128, MCOLS], bf16)
            for dst, src in [(w_t, w_d), (bias_t, bias_d), (md_t, md_d)]:
                nc.sync.dma_start(dst[:], src[:])

            rep_cm = tc.For_i(0, repeat, 1) if repeat > 1 else contextlib.nullcontext()
            with rep_cm:
              qn = 0
              for b in range(NB):
                bm = batches[b]
                nbt = bm["nbt"]
                pk_t = pkpool.tile([128, nbt * 16], i16, tag="pk")
                nc.sync.dma_start(
                    pk_t[:], pk_d[:, int(pk_off[b]):int(pk_off[b]) + nbt * 16])

                gt = gpool.tile([128, nbt * 512], bf16, tag="g")
                for h, gl, rel, nbh in bm["fills"]:
                    src = xcat[:] if h == 0 else xcat[HALF:, :]
                    nc.gpsimd.dma_gather(
                        gt[:, rel * 512:(rel + nbh) * 512]
                          .rearrange("p (b e) -> p b e", e=512),
                        src,
                        pk_t[:, rel * 8:(rel + nbh) * 8],
                        nbh * 128, nbh * 128, 512,
                        queue_num=qn,
                    )
                    qn = (qn + 1) % NQ

                v_t = vpool.tile([128, nbt * MCOLS], bf16, tag="v")
                jl_ap = pk_t[:, nbt * 14:nbt * 16].bitcast(f32)
                c6_ap = pk_t[:, nbt * 8:nbt * 14].bitcast(bf16)
                for blk in range(nbt):
                    nc.vector.tensor_scalar(
                        v_t[:, blk * MCOLS:(blk + 1) * MCOLS], md_t[:],
                        jl_ap[:, blk:blk + 1], None,
                        mybir.AluOpType.is_equal)
                    c6rep = c6_ap[:, blk * 6:blk * 6 + 6] \
                        .unsqueeze(2).broadcast_to((128, 6, GR))
                    nc.vector.tensor_tensor(
                        v_t[:, blk * MCOLS:(blk + 1) * MCOLS]
                            .rearrange("p (s x) -> p s x", x=GR),
                        v_t[:, blk * MCOLS:(blk + 1) * MCOLS]
                            .rearrange("p (s x) -> p s x", x=GR),
                        c6rep, mybir.AluOpType.mult)

                pbuf = pbpool.tile([128, GPB * 504], f32, tag="pbuf")
                for gl in range(GPB):
                    blocks = bm["gblocks"][gl]
                    if not blocks:
                        nc.vector.memset(
                            pbuf[:].rearrange(
                                "p (q s g j) -> p q s g j", q=4, s=6, g=GPB)[
                                :, :, :, gl, :], 0.0)
                        continue
                    p_t = pspool.tile([128, 504], f32, tag="p")
                    nlast = len(blocks) - 1
                    for q in range(4):
                        for i, blk in enumerate(blocks):
                            nc.tensor.matmul(
                                p_t[:, q * 126:(q + 1) * 126],
                                gt[:, blk * 512 + q * 128:blk * 512 + (q + 1) * 128],
                                v_t[:, blk * 126:(blk + 1) * 126],
                                start=(i == 0), stop=(i == nlast))
                    pb_dst = pbuf[:].rearrange(
                        "p (q s g j) -> p q s g j", q=4, s=6, g=GPB)[
                        :, :, :, gl, :]
                    nc.scalar.copy(pb_dst, p_t[:])

                po_r = popool.tile([128, C], f32, tag="por")
                po_i = popool.tile([128, C], f32, tag="poi")
                nmm = {0: 0, 1: 0}
                for tgt, q, s, wi in mms:
                    po = po_r if tgt == 0 else po_i
                    plane = q * 6 + s
                    lhsT = pbuf[:, plane * MCOLS:(plane + 1) * MCOLS]
                    nc.tensor.matmul(
                        po[:MCOLS, :], lhsT, w_t[:, wi * C:(wi + 1) * C],
                        start=(nmm[tgt] == 0), stop=(nmm[tgt] == 11))
                    nmm[tgt] += 1
                o_r = ospool.tile([128, C], f32, tag="or")
                o_i = ospool.tile([128, C], f32, tag="oi")
                nc.vector.tensor_tensor(o_r[:MCOLS, :], po_r[:MCOLS, :],
                                        bias_t[:MCOLS, :], mybir.AluOpType.add)
                nc.vector.tensor_tensor(o_i[:MCOLS, :], po_i[:MCOLS, :],
                                        bias_t[:MCOLS, :], mybir.AluOpType.add)
                nc.sync.dma_start(or_d[b * ROWS_PB:(b + 1) * ROWS_PB, :],
                                   o_r[:MCOLS, :])
                nc.sync.dma_start(oi_d[b * ROWS_PB:(b + 1) * ROWS_PB, :],
                                  o_i[:MCOLS, :])


def _make_nc(prep, repeat=1):
    import concourse.bacc as bacc
    nc = bacc.Bacc("TRN2", target_bir_lowering=False, debug=False,
                   num_swdge_queues=NQ)
    _build(nc, prep, repeat=repeat)
    nc.compile()
    return nc


def _in_maps(prep, X_real, X_imag):
    xcat = _bf16(np.concatenate(
        [np.asarray(X_real, np.float32), np.asarray(X_imag, np.float32)],
        axis=1))
    maps = []
    for c in range(CORES):
        maps.append({
            "xcat": xcat, "pk": prep["per_core"][c]["pk"],
            "wt": prep["wsb"], "biasr": prep["biasr"], "mdiv6": prep["mdiv6"],
        })
    return maps


def kernel(X_real, X_imag, L_real_vals, L_imag_vals, weight, bias, rows, cols):
    from concourse.bass_utils import run_bass_kernel_spmd

    prep = _preprocess(rows, cols, L_real_vals, L_imag_vals, weight, bias)
    nc = _make_nc(prep)
    res = run_bass_kernel_spmd(nc, _in_maps(prep, X_real, X_imag),
                               core_ids=list(range(CORES)))
    out_r = np.concatenate([res.results[c]["out_r"][:RPC] for c in range(CORES)], 0)
    out_i = np.concatenate([res.results[c]["out_i"][:RPC] for c in range(CORES)], 0)
    return out_r, out_i


# revision 10
# speedup vs baseline: 1.1877x; 1.1358x over previous
"""ChebConv (complex, K+1=3 hops) Trainium2 kernel over 8 NeuronCores.

Sharding: 1D node partition on destination rows (6250 rows/core), full X
replicated in each core's HBM; each core processes exactly the edges
targeting its rows, so no collectives are needed.

Per core, edges are bucketed by (batch of 6 row-groups, col-half, group)
and padded to 128-edge blocks with a layout shared across cores (max block
count per bucket). Per batch bt:

  pk   = one packed [128, nbt*15] int16 stream: gather indices (wrapped
         16-part layout, 8 cols/block), c6 vals (6 bf16/block) and local
         row jl (1 bf16/block)
  gt   = TWO dma_gather calls (one per col half) fetch all nbt blocks'
         neighbor rows ([128, 512] bf16 each) in one go — the ~1us fixed
         SWDGE cost is paid 2x per batch instead of 2x per group
  V    = built for the whole batch in 2 DVE ops (broadcast is_equal
         against the j-iota, broadcast multiply by c6)
  P_g  = per group, 4 PE matmul accumulation chains (feature chunks q)
         over the group's blocks: P[q] += G_chunk.T @ V_block
  out  = per batch, 24 PE matmuls contract P with signed weight tiles;
         bias added during PSUM->SBUF copy on DVE
"""
import sys
sys.path.insert(0, '/opt/trn_rl_repo')

import numpy as np
import ml_dtypes

N = 50000
E = 1_600_000
K1 = 3
C = 256
CORES = 8
RPC = N // CORES            # 6250 rows per core
GR = 21                     # rows per group
MCOLS = 6 * GR              # 126 one-hot columns
GPB = 6                     # groups per batch
ROWS_PB = GR * GPB          # 126
NB = -(-RPC // ROWS_PB)     # 50 batches
REAL_GRP = -(-RPC // GR)    # 298
HALF = 32768
NQ = 4                      # SWDGE queues


def _bf16(x):
    return x.astype(ml_dtypes.bfloat16)


def _preprocess(rows, cols, Lr, Li, weight, bias):
    rows = np.asarray(rows).astype(np.int64)
    cols = np.asarray(cols).astype(np.int64)
    core = rows // RPC
    rloc = rows - core * RPC
    g = rloc // GR
    bt = g // GPB
    gl = g - bt * GPB
    jl = (rloc - g * GR).astype(np.float32)

    C6 = np.empty((E, 6), np.float32)
    C6[:, 0:3] = np.asarray(Lr).T
    C6[:, 3:6] = np.asarray(Li).T

    colh = (cols >= HALF).astype(np.int64)
    key = ((core * NB + bt) * 2 + colh) * GPB + gl
    order = np.argsort(key, kind="stable")
    key_s = key[order]
    nbuck = CORES * NB * 2 * GPB
    bounds = np.searchsorted(key_s, np.arange(nbuck + 1))
    cnt = (bounds[1:] - bounds[:-1]).reshape(CORES, NB, 2, GPB)

    # shared block counts per (batch, half, group) = max over cores
    nblk = -(-cnt.max(axis=0) // 128)             # [NB, 2, GPB]
    for b in range(NB):
        for l in range(GPB):
            if b * GPB + l < REAL_GRP and nblk[b, :, l].sum() == 0:
                nblk[b, 0, l] = 1

    nbt = nblk.sum(axis=(1, 2))                   # blocks per batch
    tot_blk = int(nbt.sum())
    # per-batch metadata
    batches = []                                  # per bt: dict
    pk_off = np.zeros(NB + 1, np.int64)
    blk_abs = 0
    for b in range(NB):
        pk_off[b + 1] = pk_off[b] + nbt[b] * 16
        halves = []                               # (h, rel_off, nbh)
        gblocks = [[] for _ in range(GPB)]        # rel block ids per group
        fills = []                                # (h, gl, rel_b, nb)
        rel = 0
        for h in range(2):
            nbh = int(nblk[b, h].sum())
            if nbh:
                halves.append((h, rel, nbh))
            for l in range(GPB):
                nbl = int(nblk[b, h, l])
                if nbl:
                    gblocks[l].extend(range(rel, rel + nbl))
                    fills.append((h, l, rel, nbl))
                    rel += nbl
        assert rel == nbt[b]
        batches.append(dict(nbt=int(nbt[b]), halves=halves,
                            gblocks=gblocks, fills=fills, abs0=blk_abs))
        blk_abs += int(nbt[b])
    tot_pack = int(pk_off[NB])

    cols_s = cols[order]
    C6_s = _bf16(C6[order]).view(np.uint16)
    jl_s = jl[order].astype(np.float32).reshape(-1, 1).view(np.uint16)
    per_core = []
    for c in range(CORES):
        pk = np.zeros((128, tot_pack), np.uint16)
        for b in range(NB):
            nb_b = batches[b]["nbt"]
            base = int(pk_off[b])
            idx16 = np.zeros(nb_b * 128, np.int16)
            c6t = np.zeros((128, nb_b * 6), np.uint16)
            jlf = np.zeros((128, nb_b * 2), np.uint16)
            for h, l, rel_b, nbl in batches[b]["fills"]:
                buck = ((c * NB + b) * 2 + h) * GPB + l
                lo, hi = int(bounds[buck]), int(bounds[buck + 1])
                ne = hi - lo
                if ne == 0:
                    continue
                idx16[rel_b * 128:rel_b * 128 + ne] = \
                    (cols_s[lo:hi] - h * HALF).astype(np.int16)
                cc = C6_s[lo:hi]
                jj = jl_s[lo:hi]
                for k in range(nbl):
                    a, e = k * 128, min((k + 1) * 128, ne)
                    if a >= e:
                        break
                    c6t[0:e - a, (rel_b + k) * 6:(rel_b + k) * 6 + 6] = cc[a:e]
                    jlf[0:e - a, (rel_b + k) * 2:(rel_b + k) * 2 + 2] = jj[a:e]
            idxw = np.tile(idx16.reshape(-1, 16).T, (8, 1))  # [128, nb_b*8]
            pk[:, base:base + nb_b * 8] = idxw.view(np.uint16)
            pk[:, base + nb_b * 8:base + nb_b * 14] = c6t
            pk[:, base + nb_b * 14:base + nb_b * 16] = jlf
        per_core.append(dict(pk=np.ascontiguousarray(pk.view(np.int16))))

    # weight tiles [12][128, 256] f32: 0..5 = +W[k][fh], 6..11 = -W[k][fh]
    weight = np.asarray(weight, np.float32)
    wt = np.empty((12, 128, C), np.float32)
    for fh in range(2):
        for k in range(K1):
            wt[fh * 3 + k] = weight[k][fh * 128:(fh + 1) * 128]
            wt[6 + fh * 3 + k] = -weight[k][fh * 128:(fh + 1) * 128]
    wsb = np.ascontiguousarray(_bf16(wt.transpose(1, 0, 2).reshape(128, 12 * C)))

    biasr = np.ascontiguousarray(np.tile(np.asarray(bias, np.float32), (128, 1)))
    # V column layout per block: m = s*21 + j  ->  j = m % 21
    mdiv6 = np.ascontiguousarray(
        _bf16(np.tile((np.arange(MCOLS) % GR).astype(np.float32), (128, 1))))

    return dict(tot_pack=tot_pack, pk_off=pk_off, batches=batches,
                per_core=per_core, wsb=wsb, biasr=biasr, mdiv6=mdiv6)


def _final_mm_list():
    """(target, q, s, wtile): target 0=real 1=imag; q = P region; s = slot."""
    mms = []
    for tgt in range(2):
        for fh in range(2):
            for k in range(K1):
                if tgt == 0:
                    mms.append((0, fh, k, fh * 3 + k))               # +W  P_r
                    mms.append((0, 2 + fh, 3 + k, 6 + fh * 3 + k))   # -W  P_i
                else:
                    mms.append((1, fh, 3 + k, fh * 3 + k))           # +W  P_r
                    mms.append((1, 2 + fh, k, fh * 3 + k))           # +W  P_i
    return mms


def _build(nc, prep, repeat=1):
    import concourse.mybir as mybir
    from concourse.tile import TileContext
    import contextlib

    f32 = mybir.dt.float32
    bf16 = mybir.dt.bfloat16
    i16 = mybir.dt.int16
    tot_pack = prep["tot_pack"]
    pk_off = prep["pk_off"]
    batches = prep["batches"]

    xcat = nc.dram_tensor("xcat", [N, 512], bf16, kind="ExternalInput")
    pk_d = nc.dram_tensor("pk", [128, tot_pack], i16, kind="ExternalInput")
    w_d = nc.dram_tensor("wt", [128, 12 * C], bf16, kind="ExternalInput")
    bias_d = nc.dram_tensor("biasr", [128, C], f32, kind="ExternalInput")
    md_d = nc.dram_tensor("mdiv6", [128, MCOLS], bf16, kind="ExternalInput")
    or_d = nc.dram_tensor("out_r", [NB * ROWS_PB, C], f32, kind="ExternalOutput")
    oi_d = nc.dram_tensor("out_i", [NB * ROWS_PB, C], f32, kind="ExternalOutput")

    mms = _final_mm_list()

    with TileContext(nc) as tc:
        with tc.tile_pool(name="const", bufs=1) as cpool, \
             tc.tile_pool(name="pk", bufs=3) as pkpool, \
             tc.tile_pool(name="g", bufs=2) as gpool, \
             tc.tile_pool(name="v", bufs=2) as vpool, \
             tc.tile_pool(name="pb", bufs=2) as pbpool, \
             tc.tile_pool(name="os", bufs=4) as ospool, \
             tc.tile_pool(name="ps", bufs=4, space="PSUM") as pspool, \
             tc.tile_pool(name="po", bufs=2, space="PSUM") as popool:

            w_t = cpool.tile([128, 12 * C], bf16)
            bias_t = cpool.tile([128, C], f32)
            md_t = cpool.tile([128, MCOLS], bf16)
            for dst, src in [(w_t, w_d), (bias_t, bias_d), (md_t, md_d)]:
                nc.sync.dma_start(dst[:], src[:])

            rep_cm = tc.For_i(0, repeat, 1) if repeat > 1 else contextlib.nullcontext()
            with rep_cm:
              qn = 0
              for b in range(NB):
                bm = batches[b]
                nbt = bm["nbt"]
                pk_t = pkpool.tile([128, nbt * 16], i16, tag="pk")
                nc.sync.dma_start(
                    pk_t[:], pk_d[:, int(pk_off[b]):int(pk_off[b]) + nbt * 16])

                gt = gpool.tile([128, nbt * 512], bf16, tag="g")
                for h, rel, nbh in bm["halves"]:
                    src = xcat[:] if h == 0 else xcat[HALF:, :]
                    nc.gpsimd.dma_gather(
                        gt[:, rel * 512:(rel + nbh) * 512]
                          .rearrange("p (b e) -> p b e", e=512),
                        src,
                        pk_t[:, rel * 8:(rel + nbh) * 8],
                        nbh * 128, nbh * 128, 512,
                        queue_num=qn, single_packet=False,
                    )
                    qn = (qn + 1) % NQ

                v_t = vpool.tile([128, nbt * MCOLS], bf16, tag="v")
                jl_ap = pk_t[:, nbt * 14:nbt * 16].bitcast(f32)
                c6_ap = pk_t[:, nbt * 8:nbt * 14].bitcast(bf16)
                for blk in range(nbt):
                    c6rep = c6_ap[:, blk * 6:blk * 6 + 6] \
                        .unsqueeze(2).broadcast_to((128, 6, GR))
                    nc.vector.scalar_tensor_tensor(
                        v_t[:, blk * MCOLS:(blk + 1) * MCOLS]
                            .rearrange("p (s x) -> p s x", x=GR),
                        md_t[:].rearrange("p (s x) -> p s x", x=GR),
                        jl_ap[:, blk:blk + 1],
                        c6rep,
                        mybir.AluOpType.is_equal, mybir.AluOpType.mult)

                pbuf = pbpool.tile([128, GPB * 504], bf16, tag="pbuf")
                for gl in range(GPB):
                    blocks = bm["gblocks"][gl]
                    if not blocks:
                        nc.vector.memset(
                            pbuf[:].rearrange(
                                "p (q s g j) -> p q s g j", q=4, s=6, g=GPB)[
                                :, :, :, gl, :], 0.0)
                        continue
                    p_t = pspool.tile([128, 504], f32, tag="p")
                    nlast = len(blocks) - 1
                    for q in range(4):
                        for i, blk in enumerate(blocks):
                            nc.tensor.matmul(
                                p_t[:, q * 126:(q + 1) * 126],
                                gt[:, blk * 512 + q * 128:blk * 512 + (q + 1) * 128],
                                v_t[:, blk * 126:(blk + 1) * 126],
                                start=(i == 0), stop=(i == nlast))
                    pb_dst = pbuf[:].rearrange(
                        "p (q s g j) -> p q s g j", q=4, s=6, g=GPB)[
                        :, :, :, gl, :]
                    nc.scalar.copy(pb_dst, p_t[:])

                po_r = popool.tile([128, C], f32, tag="por")
                po_i = popool.tile([128, C], f32, tag="poi")
                nmm = {0: 0, 1: 0}
                for tgt, q, s, wi in mms:
                    po = po_r if tgt == 0 else po_i
                    plane = q * 6 + s
                    lhsT = pbuf[:, plane * MCOLS:(plane + 1) * MCOLS]
                    nc.tensor.matmul(
                        po[:MCOLS, :], lhsT, w_t[:, wi * C:(wi + 1) * C],
                        start=(nmm[tgt] == 0), stop=(nmm[tgt] == 11))
                    nmm[tgt] += 1
                o_r = ospool.tile([128, C], f32, tag="or")
                o_i = ospool.tile([128, C], f32, tag="oi")
                nc.vector.tensor_tensor(o_r[:MCOLS, :], po_r[:MCOLS, :],
                                        bias_t[:MCOLS, :], mybir.AluOpType.add)
                nc.vector.tensor_tensor(o_i[:MCOLS, :], po_i[:MCOLS, :],
                                        bias_t[:MCOLS, :], mybir.AluOpType.add)
                nc.sync.dma_start(or_d[b * ROWS_PB:(b + 1) * ROWS_PB, :],
                                   o_r[:MCOLS, :])
                nc.sync.dma_start(oi_d[b * ROWS_PB:(b + 1) * ROWS_PB, :],
                                  o_i[:MCOLS, :])


def _make_nc(prep, repeat=1):
    import concourse.bacc as bacc
    nc = bacc.Bacc("TRN2", target_bir_lowering=False, debug=False,
                   num_swdge_queues=NQ)
    _build(nc, prep, repeat=repeat)
    nc.compile()
    return nc


def _in_maps(prep, X_real, X_imag):
    xcat = _bf16(np.concatenate(
        [np.asarray(X_real, np.float32), np.asarray(X_imag, np.float32)],
        axis=1))
    maps = []
    for c in range(CORES):
        maps.append({
            "xcat": xcat, "pk": prep["per_core"][c]["pk"],
            "wt": prep["wsb"], "biasr": prep["biasr"], "mdiv6": prep["mdiv6"],
        })
    return maps


def kernel(X_real, X_imag, L_real_vals, L_imag_vals, weight, bias, rows, cols):
    from concourse.bass_utils import run_bass_kernel_spmd

    prep = _preprocess(rows, cols, L_real_vals, L_imag_vals, weight, bias)
    nc = _make_nc(prep)
    res = run_bass_kernel_spmd(nc, _in_maps(prep, X_real, X_imag),
                               core_ids=list(range(CORES)))
    out_r = np.concatenate([res.results[c]["out_r"][:RPC] for c in range(CORES)], 0)
    out_i = np.concatenate([res.results[c]["out_i"][:RPC] for c in range(CORES)], 0)
    return out_r, out_i
